# revision 1
# baseline (speedup 1.0000x reference)
"""2-layer LSTM encoder (B=32, T=512, F=H=1024) on 8 TRN2 NeuronCores.

Strategy: the serial recurrence dominates, so the recurrent weight matrices
U0/U1 are column-split 8 ways (each core owns 128 h-columns = a 512-column
gate slice). Every step each core computes its z^T slice with 32 bf16
matmuls, applies gates locally, and broadcasts its [128,32] h-tile to all 8
cores via remote_dma_broadcast (SBUF->SBUF, XOR-relative dests, register
slot offset). The two layers run interleaved with a LAG so one combined
[128,64] bf16 send per superstep carries h0_t and h1_{t-LAG}; layer-1 input
projections (xz1 = seq0 @ W1) are chunk-GEMMed inline 4 matmuls/superstep.
Input projections for layer 0 run as a chunked GEMM phase before the loop.
"""
import sys
for _p in ("/opt/trn_rl_repo",):
    if _p not in sys.path:
        sys.path.insert(0, _p)

import numpy as np
import ml_dtypes
import concourse.bass as bass
from concourse import mybir

F32 = mybir.dt.float32
BF16 = mybir.dt.bfloat16
SIG = mybir.ActivationFunctionType.Sigmoid
TANH = mybir.ActivationFunctionType.Tanh

NC = 8
B = 32
KT = 8
G = 4
P = 128
SW = 64          # combined send width (h0 | h1), 128B lines
CH = 16          # steps per xz chunk
GB = G * B       # 128: one step's z row-block
RD_ALL = [(0, k) for k in range(NC)]
PERM = [0, 1, 3, 2]   # storage gate order: i, f, o, g
CI, CF, CO, CG = 0, B, 2 * B, 3 * B  # column starts in z/gate tiles


def build(T, LAG=32):
    assert LAG < T and T % CH == 0
    S = T + LAG
    NCH = T // CH
    from concourse import bacc
    nc = bacc.Bacc("TRN2", target_bir_lowering=False, debug=False, num_devices=NC)

    # ---------------- DRAM ----------------
    xT_d = nc.dram_tensor("xT", [KT, P, T * B], BF16, kind="ExternalInput")
    W0_d = nc.dram_tensor("W0L", [P, KT, G * P], BF16, kind="ExternalInput")
    U0_d = nc.dram_tensor("U0L", [P, KT, G * P], BF16, kind="ExternalInput")
    W1_d = nc.dram_tensor("W1L", [P, KT, G * P], BF16, kind="ExternalInput")
    U1_d = nc.dram_tensor("U1L", [P, KT, G * P], BF16, kind="ExternalInput")
    b0_d = nc.dram_tensor("b0L", [P, G], F32, kind="ExternalInput")
    b1_d = nc.dram_tensor("b1L", [P, G], F32, kind="ExternalInput")
    h1_o = nc.dram_tensor("h1_out", [P, B], F32, kind="ExternalOutput")
    c1_o = nc.dram_tensor("c1_out", [P, B], F32, kind="ExternalOutput")

    xz0_d = nc.dram_tensor("xz0d", [NCH, P, CH * GB], F32)
    xz1_d = nc.dram_tensor("xz1d", [NCH, P, CH * GB], F32)
    seq_d = nc.dram_tensor("seqd", [KT, P, T * B], BF16)

    # ---------------- SBUF ----------------
    sb = nc.alloc_sbuf_tensor
    W0 = sb("W0s", [P, KT, G * P], BF16)
    U0 = sb("U0s", [P, KT, G * P], BF16)
    W1 = sb("W1s", [P, KT, G * P], BF16)
    U1 = sb("U1s", [P, KT, G * P], BF16)
    b0 = sb("b0s", [P, G], F32)
    b1 = sb("b1s", [P, G], F32)
    xtbuf = [sb(f"xtb{i}", [P, KT, 512], BF16) for i in range(2)]
    seqin = [sb(f"sqi{i}", [P, KT, 512], BF16) for i in range(2)]
    stg0 = [sb(f"stg0{i}", [P, CH, GB], F32) for i in range(2)]
    stg1 = [sb(f"stg1{i}", [P, CH, GB], F32) for i in range(2)]
    xz0b = [sb(f"xz0b{i}", [P, CH, GB], F32) for i in range(2)]
    xz1b = [sb(f"xz1b{i}", [P, CH, GB], F32) for i in range(2)]
    hTc = [sb(f"hTc{i}", [P, SW], BF16) for i in range(2)]
    gath = [sb(f"gat{i}", [P, NC * SW], BF16) for i in range(2)]
    seqstg = [sb(f"sqs{i}", [P, NC * B], BF16) for i in range(2)]
    zsb = [[sb(f"z{l}_{i}", [P, GB], F32) for i in range(2)] for l in range(2)]
    gsb = [[sb(f"g{l}_{i}", [P, GB], F32) for i in range(2)] for l in range(2)]
    tcb = [[sb(f"tc{l}_{i}", [P, B], F32) for i in range(2)] for l in range(2)]
    cst = [sb(f"c{l}", [P, B], F32) for l in range(2)]
    hf1 = sb("hf1", [P, B], F32)

    # ---------------- PSUM: 2x2 recurrence banks + 4 phase banks = 8 ----------------
    psL = [[nc.alloc_psum_tensor(f"psL{l}_{i}", [P, GB], F32) for i in range(2)]
           for l in range(2)]
    psA = [nc.alloc_psum_tensor(f"psA{g}", [P, 512], F32) for g in range(G)]

    # ---------------- semaphores ----------------
    sem = nc.alloc_semaphore
    rsem = [sem(f"rsem{i}") for i in range(2)]
    lsem = [sem(f"lsem{i}") for i in range(2)]
    prep = sem("prep")
    h0sem, h1sem = sem("h0sem"), sem("h1sem")
    pesem = [sem(f"pesem{l}") for l in range(2)]
    zgsem = [sem(f"zgsem{l}") for l in range(2)]   # +1 per layer-step (merged z-add)
    asem = [sem(f"asem{l}") for l in range(2)]     # +2 per layer-step (sig96 + tanh32)
    crsem = [sem(f"crsem{l}") for l in range(2)]
    tcsem = [sem(f"tcsem{l}") for l in range(2)]
    dvself = [sem(f"dvself{l}") for l in range(2)]
    windma = sem("windma")
    xta, xt1a = sem("xta"), sem("xt1a")
    xz0a, xz1a = sem("xz0a"), sem("xz1a")
    xz0wr, xz1wr = sem("xz0wr"), sem("xz1wr")
    A0pe, A0dv = sem("A0pe"), sem("A0dv")
    A1pe, A1dv = sem("A1pe"), sem("A1dv")
    seqdma = sem("seqdma")
    seqcp = sem("seqcp")
    outdma = sem("outdma")

    # A' schedule: chunk j -> 4 MMs per superstep over [16j+18, 16j+26);
    # bias-adds at w in {1,3,5,7}; out-DMA at 16j+26; xz1 prefetch at 16j+28.
    def ap_chunk_of(t):
        if t < 18:
            return None
        j = (t - 18) // CH
        w = (t - 18) % CH
        if j >= NCH or w >= 8:
            return None
        return j, w

    with nc.Block() as block:

        # ================= SYNC =================
        @block.sync
        def _(sync):
            for d, s_ in ((W0_d, W0), (U0_d, U0), (W1_d, W1), (U1_d, U1),
                          (b0_d, b0), (b1_d, b1)):
                sync.dma_start(out=s_[:], in_=d[:, :]).then_inc(windma, 16)
            # phase A in/out
            for j in range(NCH):
                if j >= 2:
                    sync.wait_ge(A0pe, G * (j - 1))
                if j >= 1:
                    sync.wait_ge(xta, 16 * j)
                sync.dma_start(
                    out=xtbuf[j % 2][:],
                    in_=xT_d[:, :, 512 * j:512 * (j + 1)].rearrange("k p n -> p k n"),
                ).then_inc(xta, 16)
                if j >= 1:
                    jo = j - 1
                    sync.wait_ge(A0dv, G * (jo + 1))
                    if jo >= 1:
                        sync.wait_ge(xz0wr, 16 * jo)
                    sync.dma_start(out=xz0_d[jo], in_=stg0[jo % 2][:]).then_inc(xz0wr, 16)
            sync.wait_ge(A0dv, G * NCH)
            sync.wait_ge(xz0wr, 16 * (NCH - 1))
            sync.dma_start(out=xz0_d[NCH - 1], in_=stg0[(NCH - 1) % 2][:]).then_inc(xz0wr, 16)
            sync.wait_ge(xz0wr, 16)
            sync.dma_start(out=xz0b[0][:], in_=xz0_d[0]).then_inc(xz0a, 16)

            # ---- loop phase ----
            for t in range(S):
                u = t - 1
                if 0 <= u < T:
                    sync.wait_ge(seqcp, u + 1)
                    if u >= 1:
                        sync.wait_ge(seqdma, 16 * u)
                    sync.dma_start(
                        out=seq_d[:, :, B * u:B * (u + 1)].rearrange("k p n -> p k n"),
                        in_=seqstg[u % 2].rearrange("p (k w) -> p k w", k=NC),
                    ).then_inc(seqdma, 16)
                if t % CH == 0:
                    j0 = t // CH + 1
                    if j0 < NCH:
                        sync.wait_ge(zgsem[0], CH * (j0 - 1))
                        sync.wait_ge(xz0a, 16 * j0)
                        sync.dma_start(out=xz0b[j0 % 2][:], in_=xz0_d[j0]).then_inc(xz0a, 16)
                if t >= CH and (t - CH) % CH == 0:       # seqin chunk j at t=16j+16
                    j = (t - CH) // CH
                    if j < NCH:
                        sync.wait_ge(seqdma, 16 * (CH * j + CH))
                        if j >= 2:
                            sync.wait_ge(A1pe, G * (j - 1))
                        if j >= 1:
                            sync.wait_ge(xt1a, 16 * j)
                        sync.dma_start(
                            out=seqin[j % 2][:],
                            in_=seq_d[:, :, 512 * j:512 * (j + 1)].rearrange("k p n -> p k n"),
                        ).then_inc(xt1a, 16)
                if t >= 26 and (t - 26) % CH == 0:       # A' out at t=16j+26
                    j = (t - 26) // CH
                    if j < NCH:
                        sync.wait_ge(A1dv, G * (j + 1))
                        if j >= 1:
                            sync.wait_ge(xz1wr, 16 * j)
                        sync.dma_start(out=xz1_d[j], in_=stg1[j % 2][:]).then_inc(xz1wr, 16)
                if t >= 28 and (t - 28) % CH == 0:       # xz1 prefetch at t=16j+28
                    j = (t - 28) // CH
                    if j < NCH:
                        sync.wait_ge(xz1wr, 16 * (j + 1))
                        if j >= 2:
                            sync.wait_ge(zgsem[1], CH * (j - 1))
                        if j >= 1:
                            sync.wait_ge(xz1a, 16 * j)
                        sync.dma_start(out=xz1b[j % 2][:], in_=xz1_d[j]).then_inc(xz1a, 16)
            sync.wait_ge(h1sem, T + 1)
            sync.dma_start(out=h1_o[:, :], in_=hf1[:]).then_inc(outdma, 16)
            sync.dma_start(out=c1_o[:, :], in_=cst[1][:]).then_inc(outdma, 16)
            sync.wait_ge(outdma, 32)

        # ================= PE =================
        @block.tensor
        def _(pe):
            pe.wait_ge(windma, 96)
            for j in range(NCH):
                pe.wait_ge(xta, 16 * (j + 1))
                for g in range(G):
                    if j >= 1:
                        pe.wait_ge(A0dv, G * (j - 1) + g + 1)
                    for k in range(KT):
                        mm = pe.matmul(
                            psA[g][:],
                            W0[:, k, g * P:(g + 1) * P],
                            xtbuf[j % 2][:, k, :],
                            start=(k == 0), stop=(k == KT - 1),
                        )
                        if k == KT - 1:
                            mm.then_inc(A0pe, 1)
            for t in range(S):
                p, pm = t % 2, (t - 1) % 2
                if t >= 1:
                    pe.wait_ge(rsem[pm], 16 * ((t - 1) // 2 + 1))
                if 1 <= t < T:
                    if t >= 2:
                        pe.wait_ge(zgsem[0], t - 1)
                    for g in range(G):
                        for k in range(KT):
                            mm = pe.matmul(
                                psL[0][p][:, g * B:(g + 1) * B],
                                U0[:, k, g * P:(g + 1) * P],
                                gath[pm][:, k * SW:k * SW + B],
                                start=(k == 0), stop=(k == KT - 1),
                            )
                            if g == G - 1 and k == KT - 1:
                                mm.then_inc(pesem[0], 1)
                s = t - LAG
                if s >= 1:
                    if s >= 2:
                        pe.wait_ge(zgsem[1], s - 1)
                    for g in range(G):
                        for k in range(KT):
                            mm = pe.matmul(
                                psL[1][p][:, g * B:(g + 1) * B],
                                U1[:, k, g * P:(g + 1) * P],
                                gath[pm][:, k * SW + B:(k + 1) * SW],
                                start=(k == 0), stop=(k == KT - 1),
                            )
                            if g == G - 1 and k == KT - 1:
                                mm.then_inc(pesem[1], 1)
                ck = ap_chunk_of(t)
                if ck is not None:
                    j, w = ck
                    for mi in range(4 * w, 4 * w + 4):
                        g, k = mi // KT, mi % KT
                        if k == 0:
                            if mi == 0:
                                pe.wait_ge(xt1a, 16 * (j + 1))
                            if j >= 1:
                                pe.wait_ge(A1dv, G * (j - 1) + g + 1)
                        mm = pe.matmul(
                            psA[g][:],
                            W1[:, k, g * P:(g + 1) * P],
                            seqin[j % 2][:, k, :],
                            start=(k == 0), stop=(k == KT - 1),
                        )
                        if k == KT - 1:
                            mm.then_inc(A1pe, 1)

        # ================= DVE =================
        @block.vector
        def _(v):
            v.wait_ge(windma, 96)
            v.memset(hTc[0][:], 0.0)
            v.memset(hTc[1][:], 0.0)
            for j in range(NCH):
                for g in range(G):
                    v.wait_ge(A0pe, G * j + g + 1)
                    if g == 0 and j >= 2:
                        v.wait_ge(xz0wr, 16 * (j - 1))
                    v.tensor_scalar_add(
                        out=stg0[j % 2][:, :, g * B:(g + 1) * B],
                        in0=psA[g][:].rearrange("p (t b) -> p t b", b=B),
                        scalar1=b0[:, g:g + 1],
                    ).then_inc(A0dv, 1)

            def z_part(l, st, p):
                Lz = zsb[l][p]
                jj = st // CH
                if st % CH == 0:
                    v.wait_ge(xz0a if l == 0 else xz1a, 16 * (jj + 1))
                if st >= 2:
                    v.wait_ge(asem[l], 2 * (st - 1))
                if st >= 1:
                    v.wait_ge(pesem[l], st)
                xz = (xz0b if l == 0 else xz1b)[jj % 2]
                tt = st % CH
                if st >= 1:
                    v.tensor_add(out=Lz[:], in0=psL[l][p][:], in1=xz[:, tt, :],
                                 ).then_inc(zgsem[l], 1)
                else:
                    v.tensor_copy(out=Lz[:], in_=xz[:, tt, :]).then_inc(zgsem[l], 1)

            def c_part(l, st, p):
                Lg, Ltc = gsb[l][p], tcb[l][p]
                v.wait_ge(asem[l], 2 * st + 2)
                if st >= 1:
                    v.wait_ge(tcsem[l], st)
                tmp = Ltc
                v.tensor_mul(out=tmp[:], in0=Lg[:, CI:CI + B], in1=Lg[:, CG:CG + B],
                             ).then_inc(dvself[l], 1)
                if st == 0:
                    v.wait_ge(dvself[l], 1)
                    v.tensor_copy(out=cst[l][:], in_=tmp[:]).then_inc(crsem[l], 1)
                    v.sem_inc(dvself[l], 1)
                else:
                    v.wait_ge(crsem[l], st)
                    v.tensor_mul(out=cst[l][:], in0=Lg[:, CF:CF + B], in1=cst[l][:],
                                 ).then_inc(dvself[l], 1)
                    v.wait_ge(dvself[l], 2 * st + 2)
                    v.tensor_add(out=cst[l][:], in0=cst[l][:], in1=tmp[:],
                                 ).then_inc(crsem[l], 1)

            def h_part(l, st, p, sst, write_hf):
                Lg, Ltc = gsb[l][p], tcb[l][p]
                v.wait_ge(tcsem[l], st + 1)
                if sst >= 2 and (l == 0 or sst >= T):
                    v.wait_ge(lsem[p], 16 * (sst // 2))
                cols = (0, B) if l == 0 else (B, SW)
                hm = v.tensor_mul(out=hTc[p][:, cols[0]:cols[1]],
                                  in0=Lg[:, CO:CO + B], in1=Ltc[:])
                hm.then_inc(h0sem if l == 0 else h1sem, 1)
                if write_hf:
                    v.tensor_mul(out=hf1[:], in0=Lg[:, CO:CO + B], in1=Ltc[:],
                                 ).then_inc(h1sem, 1)

            for t in range(S):
                p = t % 2
                s = t - LAG
                if t < T:
                    z_part(0, t, p)
                if s >= 0:
                    z_part(1, s, p)
                if t < T:
                    c_part(0, t, p)
                if s >= 0:
                    c_part(1, s, p)
                if t < T:
                    h_part(0, t, p, t, False)
                # A' bias adds scheduled between h0 and h1
                ck = ap_chunk_of(t)
                if ck is not None:
                    j, w = ck
                    if w % 2 == 1:
                        g = (w - 1) // 2
                        v.wait_ge(A1pe, G * j + g + 1)
                        if g == 0 and j >= 2:
                            v.wait_ge(xz1wr, 16 * (j - 1))
                        v.tensor_scalar_add(
                            out=stg1[j % 2][:, :, g * B:(g + 1) * B],
                            in0=psA[g][:].rearrange("p (t b) -> p t b", b=B),
                            scalar1=b1[:, g:g + 1],
                        ).then_inc(A1dv, 1)
                if s >= 0:
                    h_part(1, s, p, t, s == T - 1)

        # ================= ACT =================
        @block.scalar
        def _(a):
            def act_layer(l, st, p):
                if st >= 2:
                    a.wait_ge(h0sem if l == 0 else h1sem, st - 1)
                a.wait_ge(zgsem[l], st + 1)
                a.activation(out=gsb[l][p][:, 0:3 * B], in_=zsb[l][p][:, 0:3 * B],
                             func=SIG).then_inc(asem[l], 1)
                a.activation(out=gsb[l][p][:, CG:CG + B], in_=zsb[l][p][:, CG:CG + B],
                             func=TANH).then_inc(asem[l], 1)
                a.wait_ge(crsem[l], st + 1)
                a.activation(out=tcb[l][p][:], in_=cst[l][:], func=TANH).then_inc(tcsem[l], 1)

            for t in range(S):
                p = t % 2
                if t < T:
                    act_layer(0, t, p)
                s = t - LAG
                if s >= 0:
                    act_layer(1, s, p)

        # ================= POOL =================
        @block.gpsimd
        def _(gp):
            pid = gp.partition_id()
            for t in range(S):
                p = t % 2
                s = t - LAG
                u = t - 1
                # PE-progress proofs for the cross-core WAR tracking
                if 1 <= t < T:
                    gp.wait_ge(pesem[0], t)
                if s >= 1:
                    gp.wait_ge(pesem[1], s)
                # stage gather u's h0 halves for the seq0 DRAM write
                if 0 <= u < T:
                    gp.wait_ge(rsem[u % 2], 16 * (u // 2 + 1))
                    if u >= 2:
                        gp.wait_ge(seqdma, 16 * (u - 1))
                    gp.tensor_copy(
                        out=seqstg[u % 2].rearrange("p (k w) -> p k w", k=NC),
                        in_=gath[u % 2].rearrange("p (k w) -> p k w", k=NC)[:, :, 0:B],
                    ).then_inc(seqcp, 1)
                    gp.wait_ge(seqcp, u + 1)  # credit own copy in the engine floor
                pr = gp.remote_dma_broadcast(
                    out_ap=gath[p][:, bass.ts(pid, SW)],
                    in_ap=hTc[p][:],
                    remote_sem=rsem[p],
                    local_sem=lsem[p],
                    rdests=RD_ALL,
                ).then_inc(prep, 1)
                if t >= 2:
                    pr._wait_ge(rsem[p], 16 * (t // 2))
                gp.wait_ge(prep, t + 1)
                if t < T:
                    gp.wait_ge(h0sem, t + 1)
                if s >= 0:
                    gp.wait_ge(h1sem, s + 1)
                gp.trigger_dma(count=1)

    nc.compile()
    return nc


# ---------------- host-side prep ----------------

def prep_w_local(W, cc):
    H = W.shape[0]
    cols = np.concatenate(
        [np.arange(g * H + cc * P, g * H + (cc + 1) * P) for g in PERM])
    WL = np.ascontiguousarray(W[:, cols]).astype(ml_dtypes.bfloat16)
    return WL.reshape(KT, P, G * P).transpose(1, 0, 2).copy()


def prep_b_local(b, cc):
    H = b.shape[0] // G
    out = np.zeros((P, G), np.float32)
    for gi, g in enumerate(PERM):
        out[:, gi] = b[g * H + cc * P:g * H + (cc + 1) * P]
    return out


def prep_xT(x):
    Bb, T, F = x.shape
    xt = np.ascontiguousarray(x.transpose(2, 1, 0)).reshape(F, T * Bb)
    return xt.reshape(KT, P, T * Bb).astype(ml_dtypes.bfloat16)


def make_in_maps(x, W0, U0, b0, W1, U1, b1):
    xT = prep_xT(x)
    maps = []
    for cc in range(NC):
        maps.append({
            "xT": xT,
            "W0L": prep_w_local(W0, cc), "U0L": prep_w_local(U0, cc),
            "W1L": prep_w_local(W1, cc), "U1L": prep_w_local(U1, cc),
            "b0L": prep_b_local(b0, cc), "b1L": prep_b_local(b1, cc),
        })
    return maps


def assemble(results):
    h1 = np.zeros((B, NC * P), np.float32)
    c1 = np.zeros((B, NC * P), np.float32)
    for cc in range(NC):
        h1[:, cc * P:(cc + 1) * P] = results[cc]["h1_out"].T
        c1[:, cc * P:(cc + 1) * P] = results[cc]["c1_out"].T
    return h1, c1


# ======================================================================
# Self-contained entry point: kernel(**inputs) -> (h1, h1, c1)
# ======================================================================
import sys as _sys

_T, _LAG = 512, 32
_nc_cache = [None]


def _get_nc():
    if _nc_cache[0] is None:
        _nc_cache[0] = build(_T, _LAG)
    return _nc_cache[0]


def kernel(x, W0, U0, b0, W1, U1, b1):
    """Full-input 2-layer LSTM encoder on 8 TRN2 NeuronCores.

    Returns (h1, h1, c1) matching the reference: layer-1 last output,
    final hidden state, final cell state — each [32, 1024] fp32.
    """
    from concourse.bass_utils import run_bass_kernel_spmd
    x = np.asarray(x, dtype=np.float32)
    W0 = np.asarray(W0, dtype=np.float32)
    U0 = np.asarray(U0, dtype=np.float32)
    b0 = np.asarray(b0, dtype=np.float32)
    W1 = np.asarray(W1, dtype=np.float32)
    U1 = np.asarray(U1, dtype=np.float32)
    b1 = np.asarray(b1, dtype=np.float32)
    nc = _get_nc()
    in_maps = make_in_maps(x, W0, U0, b0, W1, U1, b1)
    last_err = None
    for attempt in range(3):
        try:
            res = run_bass_kernel_spmd(nc, in_maps, core_ids=list(range(NC)))
            break
        except Exception as e:  # wedged device: retry once or twice
            last_err = e
            import time as _time
            _time.sleep(3)
    else:
        raise last_err
    h1, c1 = assemble(res.results)
    return (h1, h1, c1)



# revision 13
# speedup vs baseline: 5977.4386x; 5977.4386x over previous
"""2-layer LSTM encoder (B=32, T=512, F=H=1024) on 8 TRN2 NeuronCores.

Strategy: the serial recurrence dominates, so the recurrent weight matrices
U0/U1 are column-split 8 ways (each core owns 128 h-columns = a 512-column
gate slice). Every step each core computes its z^T slice with 32 bf16
matmuls, applies gates locally, and broadcasts its [128,32] h-tile to all 8
cores via remote_dma_broadcast (SBUF->SBUF, XOR-relative dests, register
slot offset). The two layers run interleaved with a LAG so one combined
[128,64] bf16 send per superstep carries h0_t and h1_{t-LAG}; layer-1 input
projections (xz1 = seq0 @ W1) are chunk-GEMMed inline 4 matmuls/superstep.
Input projections for layer 0 run as a chunked GEMM phase before the loop.
"""
import sys
for _p in ("/opt/trn_rl_repo",):
    if _p not in sys.path:
        sys.path.insert(0, _p)

import numpy as np
import ml_dtypes
import concourse.bass as bass
from concourse import mybir

F32 = mybir.dt.float32
BF16 = mybir.dt.bfloat16
SIG = mybir.ActivationFunctionType.Sigmoid
TANH = mybir.ActivationFunctionType.Tanh

NC = 8
B = 32
KT = 8
G = 4
P = 128
SW = 64          # combined send width (h0 | h1), 128B lines
CH = 16          # steps per xz chunk
GB = G * B       # 128: one step's z row-block
RD_ALL = [(0, k) for k in range(NC)]
PERM = [0, 1, 3, 2]   # storage gate order: i, f, o, g
CI, CF, CO, CG = 0, B, 2 * B, 3 * B  # column starts in z/gate tiles


def build(T, LAG=32):
    assert LAG < T and T % CH == 0
    S = T + LAG
    NCH = T // CH
    from concourse import bacc
    nc = bacc.Bacc("TRN2", target_bir_lowering=False, debug=False, num_devices=NC)

    # ---------------- DRAM ----------------
    xT_d = nc.dram_tensor("xT", [KT, P, T * B], BF16, kind="ExternalInput")
    W0_d = nc.dram_tensor("W0L", [P, KT, G * P], BF16, kind="ExternalInput")
    U0_d = nc.dram_tensor("U0L", [P, KT, G * P], BF16, kind="ExternalInput")
    W1_d = nc.dram_tensor("W1L", [P, KT, G * P], BF16, kind="ExternalInput")
    U1_d = nc.dram_tensor("U1L", [P, KT, G * P], BF16, kind="ExternalInput")
    b0_d = nc.dram_tensor("b0L", [P, G], F32, kind="ExternalInput")
    b1_d = nc.dram_tensor("b1L", [P, G], F32, kind="ExternalInput")
    h1_o = nc.dram_tensor("h1_out", [P, B], F32, kind="ExternalOutput")
    c1_o = nc.dram_tensor("c1_out", [P, B], F32, kind="ExternalOutput")

    xz0_d = nc.dram_tensor("xz0d", [NCH, P, CH * GB], F32)

    # ---------------- SBUF ----------------
    sb = nc.alloc_sbuf_tensor
    W0 = sb("W0s", [P, KT, G * P], BF16)
    U0 = sb("U0s", [P, KT, G * P], BF16)
    W1 = sb("W1s", [P, KT, G * P], BF16)
    U1 = sb("U1s", [P, KT, G * P], BF16)
    b0 = sb("b0s", [P, G], F32)
    b1 = sb("b1s", [P, G], F32)
    xtbuf = [sb(f"xtb{i}", [P, KT, 512], BF16) for i in range(2)]
    seqin = [sb(f"sqi{i}", [P, KT, 512], BF16) for i in range(2)]
    stg0 = [sb(f"stg0{i}", [P, CH, GB], F32) for i in range(2)]
    stg1 = [sb(f"stg1{i}", [P, CH, GB], F32) for i in range(3)]   # xz1 ring
    xz0b = [sb(f"xz0b{i}", [P, CH, GB], F32) for i in range(2)]
    hTc = [sb(f"hTc{i}", [P, SW], BF16) for i in range(2)]
    gath = [sb(f"gat{i}", [P, NC * SW], BF16) for i in range(2)]
    zsb = [[sb(f"z{l}_{i}", [P, GB], F32) for i in range(2)] for l in range(2)]
    gsb = [[sb(f"g{l}_{i}", [P, GB], F32) for i in range(2)] for l in range(2)]
    tcb = [[sb(f"tc{l}_{i}", [P, B], F32) for i in range(2)] for l in range(2)]
    cst = [sb(f"c{l}", [P, B], F32) for l in range(2)]
    hf1 = sb("hf1", [P, B], F32)

    # ---------------- PSUM: 2x2 recurrence banks + 4 phase banks = 8 ----------------
    psL = [[nc.alloc_psum_tensor(f"psL{l}_{i}", [P, GB], F32) for i in range(2)]
           for l in range(2)]
    psA = [nc.alloc_psum_tensor(f"psA{g}", [P, 512], F32) for g in range(G)]

    # ---------------- semaphores ----------------
    sem = nc.alloc_semaphore
    rsem = [sem(f"rsem{i}") for i in range(2)]
    lsem = [sem(f"lsem{i}") for i in range(2)]
    prep = sem("prep")
    h0sem, h1sem = sem("h0sem"), sem("h1sem")
    pesem = [sem(f"pesem{l}") for l in range(2)]
    zgsem = [sem(f"zgsem{l}") for l in range(2)]   # +1 per layer-step (merged z-add)
    asem = [sem(f"asem{l}") for l in range(2)]     # +2 per layer-step (sig96 + tanh32)
    crsem = [sem(f"crsem{l}") for l in range(2)]
    tcsem = [sem(f"tcsem{l}") for l in range(2)]
    dvself = [sem(f"dvself{l}") for l in range(2)]
    windma = sem("windma")
    xta = sem("xta")
    xz0a = sem("xz0a")
    xz0wr = sem("xz0wr")
    A0pe, A0dv = sem("A0pe"), sem("A0dv")
    A1pe, A1dv = sem("A1pe"), sem("A1dv")
    seqcp = sem("seqcp")
    outdma = sem("outdma")

    # A' schedule: chunk j -> 4 MMs per superstep over [16j+18, 16j+26);
    # bias-adds at w in {1,3,5,7}; out-DMA at 16j+26; xz1 prefetch at 16j+28.
    def ap_chunk_of(t):
        if t < 18:
            return None
        j = (t - 18) // CH
        w = (t - 18) % CH
        if j >= NCH or w >= 8:
            return None
        return j, w

    with nc.Block() as block:

        # ================= SYNC =================
        @block.sync
        def _(sync):
            for d, s_ in ((W0_d, W0), (U0_d, U0), (W1_d, W1), (U1_d, U1),
                          (b0_d, b0), (b1_d, b1)):
                sync.dma_start(out=s_[:], in_=d[:, :]).then_inc(windma, 16)
            # phase A in/out
            for j in range(NCH):
                if j >= 2:
                    sync.wait_ge(A0pe, G * (j - 1))
                if j >= 1:
                    sync.wait_ge(xta, 16 * j)
                sync.dma_start(
                    out=xtbuf[j % 2][:],
                    in_=xT_d[:, :, 512 * j:512 * (j + 1)].rearrange("k p n -> p k n"),
                ).then_inc(xta, 16)
                if j >= 1:
                    jo = j - 1
                    sync.wait_ge(A0dv, G * (jo + 1))
                    if jo >= 1:
                        sync.wait_ge(xz0wr, 16 * jo)
                    sync.dma_start(out=xz0_d[jo], in_=stg0[jo % 2][:]).then_inc(xz0wr, 16)
            sync.wait_ge(A0dv, G * NCH)
            sync.wait_ge(xz0wr, 16 * (NCH - 1))
            sync.dma_start(out=xz0_d[NCH - 1], in_=stg0[(NCH - 1) % 2][:]).then_inc(xz0wr, 16)
            sync.wait_ge(xz0wr, 16)
            sync.dma_start(out=xz0b[0][:], in_=xz0_d[0]).then_inc(xz0a, 16)

            # ---- loop phase (seq0 and xz1 stay in SBUF; only xz0 reloads) ----
            for t in range(S):
                if t % CH == 0:
                    j0 = t // CH + 1
                    if j0 < NCH:
                        sync.wait_ge(zgsem[0], CH * (j0 - 1))
                        sync.wait_ge(xz0a, 16 * j0)
                        sync.dma_start(out=xz0b[j0 % 2][:], in_=xz0_d[j0]).then_inc(xz0a, 16)
            sync.wait_ge(h1sem, T + 1)
            sync.dma_start(out=h1_o[:, :], in_=hf1[:]).then_inc(outdma, 16)
            sync.dma_start(out=c1_o[:, :], in_=cst[1][:]).then_inc(outdma, 16)
            sync.wait_ge(outdma, 32)

        # ================= PE =================
        @block.tensor
        def _(pe):
            pe.wait_ge(windma, 96)
            for j in range(NCH):
                pe.wait_ge(xta, 16 * (j + 1))
                for g in range(G):
                    if j >= 1:
                        pe.wait_ge(A0dv, G * (j - 1) + g + 1)
                    for k in range(KT):
                        mm = pe.matmul(
                            psA[g][:],
                            W0[:, k, g * P:(g + 1) * P],
                            xtbuf[j % 2][:, k, :],
                            start=(k == 0), stop=(k == KT - 1),
                        )
                        if k == KT - 1:
                            mm.then_inc(A0pe, 1)
            for t in range(S):
                p, pm = t % 2, (t - 1) % 2
                if t >= 1:
                    pe.wait_ge(rsem[pm], 16 * ((t - 1) // 2 + 1))
                if 1 <= t < T:
                    if t >= 2:
                        pe.wait_ge(zgsem[0], t - 1)
                    for g in range(G):
                        for k in range(KT):
                            mm = pe.matmul(
                                psL[0][p][:, g * B:(g + 1) * B],
                                U0[:, k, g * P:(g + 1) * P],
                                gath[pm][:, k * SW:k * SW + B],
                                start=(k == 0), stop=(k == KT - 1),
                            )
                            if g == G - 1 and k == KT - 1:
                                mm.then_inc(pesem[0], 1)
                s = t - LAG
                if s >= 1:
                    if s >= 2:
                        pe.wait_ge(zgsem[1], s - 1)
                    for g in range(G):
                        for k in range(KT):
                            mm = pe.matmul(
                                psL[1][p][:, g * B:(g + 1) * B],
                                U1[:, k, g * P:(g + 1) * P],
                                gath[pm][:, k * SW + B:(k + 1) * SW],
                                start=(k == 0), stop=(k == KT - 1),
                            )
                            if g == G - 1 and k == KT - 1:
                                mm.then_inc(pesem[1], 1)
                ck = ap_chunk_of(t)
                if ck is not None:
                    j, w = ck
                    for mi in range(4 * w, 4 * w + 4):
                        g, k = mi // KT, mi % KT
                        if k == 0:
                            if mi == 0:
                                pe.wait_ge(seqcp, CH * (j + 1))
                            if j >= 1:
                                pe.wait_ge(A1dv, G * (j - 1) + g + 1)
                        mm = pe.matmul(
                            psA[g][:],
                            W1[:, k, g * P:(g + 1) * P],
                            seqin[j % 2][:, k, :],
                            start=(k == 0), stop=(k == KT - 1),
                        )
                        if k == KT - 1:
                            mm.then_inc(A1pe, 1)

        # ================= DVE =================
        @block.vector
        def _(v):
            v.wait_ge(windma, 96)
            v.memset(hTc[0][:], 0.0)
            v.memset(hTc[1][:], 0.0)
            for j in range(NCH):
                for g in range(G):
                    v.wait_ge(A0pe, G * j + g + 1)
                    if g == 0 and j >= 2:
                        v.wait_ge(xz0wr, 16 * (j - 1))
                    v.tensor_scalar_add(
                        out=stg0[j % 2][:, :, g * B:(g + 1) * B],
                        in0=psA[g][:].rearrange("p (t b) -> p t b", b=B),
                        scalar1=b0[:, g:g + 1],
                    ).then_inc(A0dv, 1)

            def z_part(l, st, p):
                Lz = zsb[l][p]
                jj = st // CH
                if st % CH == 0:
                    if l == 0:
                        v.wait_ge(xz0a, 16 * (jj + 1))
                    else:
                        v.wait_ge(A1dv, G * (jj + 1))
                if st >= 2:
                    v.wait_ge(asem[l], 2 * (st - 1))
                if st >= 1:
                    v.wait_ge(pesem[l], st)
                xz = xz0b[jj % 2] if l == 0 else stg1[jj % 3]
                tt = st % CH
                if st >= 1:
                    v.tensor_add(out=Lz[:], in0=psL[l][p][:], in1=xz[:, tt, :],
                                 ).then_inc(zgsem[l], 1)
                else:
                    v.tensor_copy(out=Lz[:], in_=xz[:, tt, :]).then_inc(zgsem[l], 1)

            def c_part(l, st, p):
                Lg, Ltc = gsb[l][p], tcb[l][p]
                v.wait_ge(asem[l], 2 * st + 2)
                if st >= 1:
                    v.wait_ge(tcsem[l], st)
                tmp = Ltc
                v.tensor_mul(out=tmp[:], in0=Lg[:, CI:CI + B], in1=Lg[:, CG:CG + B],
                             ).then_inc(dvself[l], 1)
                if st == 0:
                    v.wait_ge(dvself[l], 1)
                    v.tensor_copy(out=cst[l][:], in_=tmp[:]).then_inc(crsem[l], 1)
                    v.sem_inc(dvself[l], 1)
                else:
                    v.wait_ge(crsem[l], st)
                    v.tensor_mul(out=cst[l][:], in0=Lg[:, CF:CF + B], in1=cst[l][:],
                                 ).then_inc(dvself[l], 1)
                    v.wait_ge(dvself[l], 2 * st + 2)
                    v.tensor_add(out=cst[l][:], in0=cst[l][:], in1=tmp[:],
                                 ).then_inc(crsem[l], 1)

            def h_part(l, st, p, sst, write_hf):
                Lg, Ltc = gsb[l][p], tcb[l][p]
                v.wait_ge(tcsem[l], st + 1)
                if sst >= 2 and (l == 0 or sst >= T):
                    v.wait_ge(lsem[p], 16 * (sst // 2))
                cols = (0, B) if l == 0 else (B, SW)
                hm = v.tensor_mul(out=hTc[p][:, cols[0]:cols[1]],
                                  in0=Lg[:, CO:CO + B], in1=Ltc[:])
                hm.then_inc(h0sem if l == 0 else h1sem, 1)
                if write_hf:
                    v.tensor_mul(out=hf1[:], in0=Lg[:, CO:CO + B], in1=Ltc[:],
                                 ).then_inc(h1sem, 1)

            for t in range(S):
                p = t % 2
                s = t - LAG
                if t < T:
                    z_part(0, t, p)
                if s >= 0:
                    z_part(1, s, p)
                if t < T:
                    c_part(0, t, p)
                if s >= 0:
                    c_part(1, s, p)
                if t < T:
                    h_part(0, t, p, t, False)
                # A' bias adds scheduled between h0 and h1
                ck = ap_chunk_of(t)
                if ck is not None:
                    j, w = ck
                    if w % 2 == 1:
                        g = (w - 1) // 2
                        v.wait_ge(A1pe, G * j + g + 1)
                        v.tensor_scalar_add(
                            out=stg1[j % 3][:, :, g * B:(g + 1) * B],
                            in0=psA[g][:].rearrange("p (t b) -> p t b", b=B),
                            scalar1=b1[:, g:g + 1],
                        ).then_inc(A1dv, 1)
                if s >= 0:
                    h_part(1, s, p, t, s == T - 1)

        # ================= ACT =================
        @block.scalar
        def _(a):
            def act_layer(l, st, p):
                if st >= 2:
                    a.wait_ge(h0sem if l == 0 else h1sem, st - 1)
                a.wait_ge(zgsem[l], st + 1)
                a.activation(out=gsb[l][p][:, 0:3 * B], in_=zsb[l][p][:, 0:3 * B],
                             func=SIG).then_inc(asem[l], 1)
                a.activation(out=gsb[l][p][:, CG:CG + B], in_=zsb[l][p][:, CG:CG + B],
                             func=TANH).then_inc(asem[l], 1)
                a.wait_ge(crsem[l], st + 1)
                a.activation(out=tcb[l][p][:], in_=cst[l][:], func=TANH).then_inc(tcsem[l], 1)

            for t in range(S):
                p = t % 2
                if t < T:
                    act_layer(0, t, p)
                s = t - LAG
                if s >= 0:
                    act_layer(1, s, p)

        # ================= POOL =================
        @block.gpsimd
        def _(gp):
            pid = gp.partition_id()
            for t in range(S):
                p = t % 2
                s = t - LAG
                u = t - 1
                # PE-progress proofs for the cross-core WAR tracking
                if 1 <= t < T:
                    gp.wait_ge(pesem[0], t)
                if s >= 1:
                    gp.wait_ge(pesem[1], s)
                # stage gather u's h0 tiles straight into the A' input window
                if 0 <= u < T:
                    gp.wait_ge(rsem[u % 2], 16 * (u // 2 + 1))
                    jj, w = u // CH, u % CH
                    if w == 0 and jj >= 2:
                        gp.wait_ge(A1pe, G * (jj - 1))
                    gp.tensor_copy(
                        out=seqin[jj % 2][:, :, w * B:(w + 1) * B],
                        in_=gath[u % 2].rearrange("p (k w) -> p k w", k=NC)[:, :, 0:B],
                    ).then_inc(seqcp, 1)
                    gp.wait_ge(seqcp, u + 1)  # credit own copy in the engine floor
                pr = gp.remote_dma_broadcast(
                    out_ap=gath[p][:, bass.ts(pid, SW)],
                    in_ap=hTc[p][:],
                    remote_sem=rsem[p],
                    local_sem=lsem[p],
                    rdests=RD_ALL,
                ).then_inc(prep, 1)
                if t >= 2:
                    pr._wait_ge(rsem[p], 16 * (t // 2))
                gp.wait_ge(prep, t + 1)
                if t < T:
                    gp.wait_ge(h0sem, t + 1)
                if s >= 0:
                    gp.wait_ge(h1sem, s + 1)
                gp.trigger_dma(count=1)

    nc.compile()
    return nc


# ---------------- host-side prep ----------------

def prep_w_local(W, cc):
    H = W.shape[0]
    cols = np.concatenate(
        [np.arange(g * H + cc * P, g * H + (cc + 1) * P) for g in PERM])
    WL = np.ascontiguousarray(W[:, cols]).astype(ml_dtypes.bfloat16)
    return WL.reshape(KT, P, G * P).transpose(1, 0, 2).copy()


def prep_b_local(b, cc):
    H = b.shape[0] // G
    out = np.zeros((P, G), np.float32)
    for gi, g in enumerate(PERM):
        out[:, gi] = b[g * H + cc * P:g * H + (cc + 1) * P]
    return out


def prep_xT(x):
    Bb, T, F = x.shape
    xt = np.ascontiguousarray(x.transpose(2, 1, 0)).reshape(F, T * Bb)
    return xt.reshape(KT, P, T * Bb).astype(ml_dtypes.bfloat16)


def make_in_maps(x, W0, U0, b0, W1, U1, b1):
    xT = prep_xT(x)
    maps = []
    for cc in range(NC):
        maps.append({
            "xT": xT,
            "W0L": prep_w_local(W0, cc), "U0L": prep_w_local(U0, cc),
            "W1L": prep_w_local(W1, cc), "U1L": prep_w_local(U1, cc),
            "b0L": prep_b_local(b0, cc), "b1L": prep_b_local(b1, cc),
        })
    return maps


def assemble(results):
    h1 = np.zeros((B, NC * P), np.float32)
    c1 = np.zeros((B, NC * P), np.float32)
    for cc in range(NC):
        h1[:, cc * P:(cc + 1) * P] = results[cc]["h1_out"].T
        c1[:, cc * P:(cc + 1) * P] = results[cc]["c1_out"].T
    return h1, c1


# ======================================================================
# Self-contained entry point: kernel(**inputs) -> (h1, h1, c1)
# ======================================================================
import sys as _sys

_T, _LAG = 512, 32
_nc_cache = [None]


def _get_nc():
    if _nc_cache[0] is None:
        _nc_cache[0] = build(_T, _LAG)
    return _nc_cache[0]


def kernel(x, W0, U0, b0, W1, U1, b1):
    """Full-input 2-layer LSTM encoder on 8 TRN2 NeuronCores.

    Returns (h1, h1, c1) matching the reference: layer-1 last output,
    final hidden state, final cell state — each [32, 1024] fp32.
    """
    from concourse.bass_utils import run_bass_kernel_spmd
    x = np.asarray(x, dtype=np.float32)
    W0 = np.asarray(W0, dtype=np.float32)
    U0 = np.asarray(U0, dtype=np.float32)
    b0 = np.asarray(b0, dtype=np.float32)
    W1 = np.asarray(W1, dtype=np.float32)
    U1 = np.asarray(U1, dtype=np.float32)
    b1 = np.asarray(b1, dtype=np.float32)
    nc = _get_nc()
    in_maps = make_in_maps(x, W0, U0, b0, W1, U1, b1)
    last_err = None
    for attempt in range(3):
        try:
            res = run_bass_kernel_spmd(nc, in_maps, core_ids=list(range(NC)))
            break
        except Exception as e:  # wedged device: retry once or twice
            last_err = e
            import time as _time
            _time.sleep(3)
    else:
        raise last_err
    h1, c1 = assemble(res.results)
    return (h1, h1, c1)

